# revision 1
# baseline (speedup 1.0000x reference)
"""Trainium2 Bass kernel for nn_Dynamic_Fusion (gnn_message_passing).

Reference computation (per batch item b):
  scores[n] = sum_{h,m} attn[b,h,n,m]            (argmax invariant to the /H mean)
  t         = argmax_n scores[n]                 (first index on ties)
  a         = depth1_ancestor(t)  in {0,1,4,7}
  update    = points[b,a] + (t!=0) * vectors[b,a-1]
  out[b,v]  = points[b,v] + Fa*update - Fa*sum_{edges e on root->v path} vectors[b,e]

Strategy: pure data parallel over 8 cores (512 batch items each), batch on
the 128 SBUF partitions (4 tiles of 128 per core), z=512 on the free dim.
The path-sum term is computed with a tree recurrence
  T[v] = T[parent(v)] - Fa*vectors[v-1],  T[0] = Fa*update
using fused scalar_tensor_tensor ops (grouped into affine node-slices),
then out = T + points in one big tensor_tensor add. The argmax is done with
reduce_max + is_ge + iota-min (replicating argsort first-max tie-breaking).
"""

import sys

for _p in ("/opt/trn_rl_repo",):
    if _p not in sys.path:
        sys.path.insert(0, _p)

from contextlib import ExitStack

import numpy as np

import concourse.bass as bass  # noqa: F401
import concourse.tile as tile
from concourse import bacc, mybir
from concourse.bass_utils import run_bass_kernel_spmd

F32 = mybir.dt.float32
ALU = mybir.AluOpType
AX = mybir.AxisListType

N_CORES = 8
B_FULL = 4096
B = B_FULL // N_CORES  # 512 batch items per core
NJ = 17  # joints
NE = 16  # edges
Z = 512
H = 8
P = 128  # SBUF partitions = batch tile
NTILES = B // P  # 4

_nc_cache = None


def _build():
    nc = bacc.Bacc("TRN2", target_bir_lowering=False, debug=False, name="dynfusion")

    pts = nc.dram_tensor("points", [B, NJ, Z], F32, kind="ExternalInput")
    vec = nc.dram_tensor("vectors", [B, NE, Z], F32, kind="ExternalInput")
    att = nc.dram_tensor("attn", [B, H, NJ, NJ], F32, kind="ExternalInput")
    fa_pos = nc.dram_tensor("fa_pos", [P, 1], F32, kind="ExternalInput")
    fa_neg = nc.dram_tensor("fa_neg", [P, 1], F32, kind="ExternalInput")
    iota = nc.dram_tensor("iota", [P, NJ], F32, kind="ExternalInput")
    out = nc.dram_tensor("out", [B, NJ, Z], F32, kind="ExternalOutput")

    stt = None  # set below (nc.vector.scalar_tensor_tensor)

    with tile.TileContext(nc) as tc, ExitStack() as ctx:
        consts = ctx.enter_context(tc.tile_pool(name="consts", bufs=1))
        p_pool = ctx.enter_context(tc.tile_pool(name="pts", bufs=2))
        v_pool = ctx.enter_context(tc.tile_pool(name="vec", bufs=2))
        a_pool = ctx.enter_context(tc.tile_pool(name="attn", bufs=2))
        t_pool = ctx.enter_context(tc.tile_pool(name="tbuf", bufs=1))
        s_pool = ctx.enter_context(tc.tile_pool(name="small", bufs=2))

        stt = nc.vector.scalar_tensor_tensor

        fa_p = consts.tile([P, 1], F32)
        nc.sync.dma_start(fa_p[:], fa_pos.ap())
        fa_n = consts.tile([P, 1], F32)
        nc.sync.dma_start(fa_n[:], fa_neg.ap())
        io = consts.tile([P, NJ], F32)
        nc.sync.dma_start(io[:], iota.ap())

        for it in range(NTILES):
            r0 = it * P

            A = a_pool.tile([P, H, NJ, NJ], F32)
            nc.sync.dma_start(A[:], att.ap()[r0 : r0 + P])
            V = v_pool.tile([P, NE, Z], F32)
            nc.sync.dma_start(V[:], vec.ap()[r0 : r0 + P])
            Pt = p_pool.tile([P, NJ, Z], F32)
            nc.sync.dma_start(Pt[:], pts.ap()[r0 : r0 + P])

            # --- scores[n] = sum over (h, m): one XY-reduce on [p, n, h, m] view
            sc = s_pool.tile([P, NJ], F32)
            nc.vector.tensor_reduce(
                sc[:], A[:].rearrange("p h n m -> p n h m"), axis=AX.XY, op=ALU.add
            )
            # --- argmax with first-index tie-break
            mx = s_pool.tile([P, 1], F32)
            nc.vector.tensor_reduce(mx[:], sc[:], axis=AX.X, op=ALU.max)
            eq = s_pool.tile([P, NJ], F32)
            nc.vector.tensor_scalar(eq[:], sc[:], mx[:], None, ALU.is_ge)
            msk = s_pool.tile([P, NJ], F32)
            stt(msk[:], eq[:], -1.0e4, io[:], op0=ALU.mult, op1=ALU.add)
            tb = s_pool.tile([P, 1], F32)
            nc.vector.tensor_reduce(tb[:], msk[:], axis=AX.X, op=ALU.min)
            # tb currently = argmax - 1e4; compare against shifted thresholds
            # (avoids an extra +1e4 op): t < x  <=>  tb < x - 1e4
            c0 = s_pool.tile([P, 1], F32)
            nc.vector.tensor_scalar(c0[:], tb[:], 0.5 - 1.0e4, None, ALU.is_lt)
            c3 = s_pool.tile([P, 1], F32)
            nc.vector.tensor_scalar(c3[:], tb[:], 3.5 - 1.0e4, None, ALU.is_lt)
            c6 = s_pool.tile([P, 1], F32)
            nc.vector.tensor_scalar(c6[:], tb[:], 6.5 - 1.0e4, None, ALU.is_lt)
            s1 = s_pool.tile([P, 1], F32)
            stt(s1[:], c0[:], -1.0, c3[:], op0=ALU.mult, op1=ALU.add)  # c3-c0
            s4 = s_pool.tile([P, 1], F32)
            stt(s4[:], c3[:], -1.0, c6[:], op0=ALU.mult, op1=ALU.add)  # c6-c3
            s7 = s_pool.tile([P, 1], F32)
            nc.vector.tensor_scalar(s7[:], c6[:], -1.0, 1.0, ALU.mult, ALU.add)

            # --- update selection; use T rows 1..4 as scratch for the 4
            # scaled point candidates (they are overwritten by the chain later)
            T = t_pool.tile([P, NJ, Z], F32)
            u0 = T[:, 1, :]
            u1 = T[:, 2, :]
            u4 = T[:, 3, :]
            u7 = T[:, 4, :]
            nc.scalar.mul(u0, Pt[:, 0, :], c0[:])
            nc.scalar.mul(u1, Pt[:, 1, :], s1[:])
            nc.scalar.mul(u4, Pt[:, 4, :], s4[:])
            nc.scalar.mul(u7, Pt[:, 7, :], s7[:])
            nc.vector.tensor_add(u0, u0, u1)
            nc.vector.tensor_add(u4, u4, u7)
            nc.vector.tensor_add(u0, u0, u4)  # = selected point row
            stt(u0, V[:, 0, :], s1[:], u0, op0=ALU.mult, op1=ALU.add)
            stt(u0, V[:, 3, :], s4[:], u0, op0=ALU.mult, op1=ALU.add)
            stt(u0, V[:, 6, :], s7[:], u0, op0=ALU.mult, op1=ALU.add)
            # T[0] = Fa * update
            nc.vector.tensor_scalar(T[:, 0, :], u0, fa_p[:], None, ALU.mult)

            # --- downward tree chain: T[v] = T[parent] - Fa*V[v-1]
            # grouped into affine strided slices where parents line up
            def chain(dst, vsrc, par):
                stt(dst, vsrc, fa_n[:], par, op0=ALU.mult, op1=ALU.add)

            chain(T[:, 1, :], V[:, 0, :], T[:, 0, :])
            chain(T[:, 4, :], V[:, 3, :], T[:, 0, :])
            chain(T[:, 7, :], V[:, 6, :], T[:, 0, :])
            chain(T[:, 2:9:3, :], V[:, 1:8:3, :], T[:, 1:8:3, :])  # {2,5,8}
            chain(T[:, 3:10:3, :], V[:, 2:9:3, :], T[:, 2:9:3, :])  # {3,6,9}
            chain(T[:, 10, :], V[:, 9, :], T[:, 9, :])
            chain(T[:, 11, :], V[:, 10, :], T[:, 8, :])
            chain(T[:, 14, :], V[:, 13, :], T[:, 8, :])
            chain(T[:, 12:16:3, :], V[:, 11:15:3, :], T[:, 11:15:3, :])  # {12,15}
            chain(T[:, 13:17:3, :], V[:, 12:16:3, :], T[:, 12:16:3, :])  # {13,16}

            # --- out = T + points (in place into the points tile), then store
            pf = Pt[:].rearrange("p a b -> p (a b)")
            nc.vector.tensor_add(pf, T[:].rearrange("p a b -> p (a b)"), pf)
            nc.gpsimd.dma_start(out.ap()[r0 : r0 + P], Pt[:])

    nc.compile()
    return nc


def _get_nc():
    global _nc_cache
    if _nc_cache is None:
        _nc_cache = _build()
    return _nc_cache


def _make_in_maps(points, vectors, attntion_scors, Fa):
    points = np.ascontiguousarray(points, dtype=np.float32)
    vectors = np.ascontiguousarray(vectors, dtype=np.float32)
    attn = np.ascontiguousarray(attntion_scors, dtype=np.float32)
    fa = np.float32(np.asarray(Fa).reshape(-1)[0])
    fa_pos = np.full((P, 1), fa, dtype=np.float32)
    fa_neg = np.full((P, 1), -fa, dtype=np.float32)
    iota = np.tile(np.arange(NJ, dtype=np.float32), (P, 1))
    in_maps = []
    for c in range(N_CORES):
        s = slice(c * B, (c + 1) * B)
        in_maps.append(
            {
                "points": points[s],
                "vectors": vectors[s],
                "attn": attn[s],
                "fa_pos": fa_pos,
                "fa_neg": fa_neg,
                "iota": iota,
            }
        )
    return in_maps


def run(points, vectors, attntion_scors, Fa, trace=False, **spmd_kwargs):
    nc = _get_nc()
    in_maps = _make_in_maps(points, vectors, attntion_scors, Fa)
    res = run_bass_kernel_spmd(
        nc, in_maps, core_ids=list(range(N_CORES)), trace=trace, **spmd_kwargs
    )
    full = np.concatenate([res.results[c]["out"] for c in range(N_CORES)], axis=0)
    return full, res


def kernel(points, vectors, attntion_scors, Fa):
    full, _ = run(points, vectors, attntion_scors, Fa)
    return full


# revision 4
# speedup vs baseline: 532.4076x; 532.4076x over previous
"""Trainium2 Bass kernel for nn_Dynamic_Fusion (gnn_message_passing).

Reference computation (per batch item b):
  scores[n] = sum_{h,m} attn[b,h,n,m]            (argmax invariant to the /H mean)
  t         = argmax_n scores[n]                 (first index on ties)
  a         = depth1_ancestor(t)  in {0,1,4,7}
  update    = points[b,a] + (t!=0) * vectors[b,a-1]
  out[b,v]  = points[b,v] + Fa*update - Fa*sum_{edges e on root->v path} vectors[b,e]

Strategy: pure data parallel over 8 cores (512 batch items each), batch on
the 128 SBUF partitions (4 tiles of 128 per core), z=512 on the free dim.
The path-sum term is computed with a tree recurrence
  T[v] = T[parent(v)] - Fa*vectors[v-1],  T[0] = Fa*update
using fused scalar_tensor_tensor ops (grouped into affine node-slices),
then out = T + points in one big tensor_tensor add. The argmax is done with
reduce_max + is_ge + iota-min (replicating argsort first-max tie-breaking).
"""

import sys

for _p in ("/opt/trn_rl_repo",):
    if _p not in sys.path:
        sys.path.insert(0, _p)

from contextlib import ExitStack

import numpy as np

import concourse.bass as bass  # noqa: F401
import concourse.tile as tile
from concourse import bacc, mybir
from concourse.bass_utils import run_bass_kernel_spmd

F32 = mybir.dt.float32
ALU = mybir.AluOpType
AX = mybir.AxisListType

N_CORES = 8
B_FULL = 4096
B = B_FULL // N_CORES  # 512 batch items per core
NJ = 17  # joints
NE = 16  # edges
Z = 512
H = 8
P = 128  # SBUF partitions = batch tile
NTILES = B // P  # 4

_nc_cache = None


def _build(reps=1):
    nc = bacc.Bacc("TRN2", target_bir_lowering=False, debug=False, name="dynfusion")

    pts = nc.dram_tensor("points", [B, NJ, Z], F32, kind="ExternalInput")
    vec = nc.dram_tensor("vectors", [B, NE, Z], F32, kind="ExternalInput")
    att = nc.dram_tensor("attn", [B, H, NJ, NJ], F32, kind="ExternalInput")
    fa_pos = nc.dram_tensor("fa_pos", [P, 1], F32, kind="ExternalInput")
    fa_neg = nc.dram_tensor("fa_neg", [P, 1], F32, kind="ExternalInput")
    iota = nc.dram_tensor("iota", [P, NJ], F32, kind="ExternalInput")
    out = nc.dram_tensor("out", [B, NJ, Z], F32, kind="ExternalOutput")

    stt = None  # set below (nc.vector.scalar_tensor_tensor)

    with tile.TileContext(nc) as tc, ExitStack() as ctx:
        consts = ctx.enter_context(tc.tile_pool(name="consts", bufs=1))
        p_pool = ctx.enter_context(tc.tile_pool(name="pts", bufs=2))
        v_pool = ctx.enter_context(tc.tile_pool(name="vec", bufs=2))
        a_pool = ctx.enter_context(tc.tile_pool(name="attn", bufs=2))
        t_pool = ctx.enter_context(tc.tile_pool(name="tbuf", bufs=1))
        s_pool = ctx.enter_context(tc.tile_pool(name="small", bufs=2))

        stt = nc.vector.scalar_tensor_tensor

        fa_p = consts.tile([P, 1], F32)
        nc.sync.dma_start(fa_p[:], fa_pos.ap())
        fa_n = consts.tile([P, 1], F32)
        nc.sync.dma_start(fa_n[:], fa_neg.ap())
        io = consts.tile([P, NJ], F32)
        nc.sync.dma_start(io[:], iota.ap())

        rep_ctx = tc.For_i(0, reps, 1) if reps > 1 else None
        if rep_ctx is not None:
            rep_ctx.__enter__()

        for it in range(NTILES):
            r0 = it * P

            A = a_pool.tile([P, H, NJ, NJ], F32)
            nc.sync.dma_start(A[:], att.ap()[r0 : r0 + P])
            V = v_pool.tile([P, NE, Z], F32)
            nc.sync.dma_start(V[:], vec.ap()[r0 : r0 + P])
            Pt = p_pool.tile([P, NJ, Z], F32)
            nc.sync.dma_start(Pt[:], pts.ap()[r0 : r0 + P])

            # --- scores[n] = sum over (h, m): one XY-reduce on [p, n, h, m] view
            sc = s_pool.tile([P, NJ], F32)
            nc.vector.tensor_reduce(
                sc[:], A[:].rearrange("p h n m -> p n h m"), axis=AX.XY, op=ALU.add
            )
            # --- argmax with first-index tie-break
            mx = s_pool.tile([P, 1], F32)
            nc.vector.tensor_reduce(mx[:], sc[:], axis=AX.X, op=ALU.max)
            eq = s_pool.tile([P, NJ], F32)
            nc.vector.tensor_scalar(eq[:], sc[:], mx[:], None, ALU.is_ge)
            msk = s_pool.tile([P, NJ], F32)
            stt(msk[:], eq[:], -1.0e4, io[:], op0=ALU.mult, op1=ALU.add)
            tb = s_pool.tile([P, 1], F32)
            nc.vector.tensor_reduce(tb[:], msk[:], axis=AX.X, op=ALU.min)
            # tb currently = argmax - 1e4; compare against shifted thresholds
            # (avoids an extra +1e4 op): t < x  <=>  tb < x - 1e4
            c0 = s_pool.tile([P, 1], F32)
            nc.vector.tensor_scalar(c0[:], tb[:], 0.5 - 1.0e4, None, ALU.is_lt)
            c3 = s_pool.tile([P, 1], F32)
            nc.vector.tensor_scalar(c3[:], tb[:], 3.5 - 1.0e4, None, ALU.is_lt)
            c6 = s_pool.tile([P, 1], F32)
            nc.vector.tensor_scalar(c6[:], tb[:], 6.5 - 1.0e4, None, ALU.is_lt)
            s1 = s_pool.tile([P, 1], F32)
            stt(s1[:], c0[:], -1.0, c3[:], op0=ALU.mult, op1=ALU.add)  # c3-c0
            s4 = s_pool.tile([P, 1], F32)
            stt(s4[:], c3[:], -1.0, c6[:], op0=ALU.mult, op1=ALU.add)  # c6-c3
            s7 = s_pool.tile([P, 1], F32)
            nc.vector.tensor_scalar(s7[:], c6[:], -1.0, 1.0, ALU.mult, ALU.add)

            # --- update selection; use T rows 1..4 as scratch for the 4
            # scaled point candidates (they are overwritten by the chain later)
            T = t_pool.tile([P, NJ, Z], F32)
            u0 = T[:, 1, :]
            u1 = T[:, 2, :]
            u4 = T[:, 3, :]
            u7 = T[:, 4, :]
            nc.scalar.mul(u0, Pt[:, 0, :], c0[:])
            nc.scalar.mul(u1, Pt[:, 1, :], s1[:])
            nc.scalar.mul(u4, Pt[:, 4, :], s4[:])
            nc.scalar.mul(u7, Pt[:, 7, :], s7[:])
            nc.vector.tensor_add(u0, u0, u1)
            nc.vector.tensor_add(u4, u4, u7)
            nc.vector.tensor_add(u0, u0, u4)  # = selected point row
            stt(u0, V[:, 0, :], s1[:], u0, op0=ALU.mult, op1=ALU.add)
            stt(u0, V[:, 3, :], s4[:], u0, op0=ALU.mult, op1=ALU.add)
            stt(u0, V[:, 6, :], s7[:], u0, op0=ALU.mult, op1=ALU.add)
            # T[0] = Fa * update
            nc.vector.tensor_scalar(T[:, 0, :], u0, fa_p[:], None, ALU.mult)

            # --- downward tree chain: T[v] = T[parent] - Fa*V[v-1]
            # grouped into affine strided slices where parents line up
            def chain(dst, vsrc, par):
                stt(dst, vsrc, fa_n[:], par, op0=ALU.mult, op1=ALU.add)

            chain(T[:, 1, :], V[:, 0, :], T[:, 0, :])
            chain(T[:, 4, :], V[:, 3, :], T[:, 0, :])
            chain(T[:, 7, :], V[:, 6, :], T[:, 0, :])
            chain(T[:, 2:9:3, :], V[:, 1:8:3, :], T[:, 1:8:3, :])  # {2,5,8}
            chain(T[:, 3:10:3, :], V[:, 2:9:3, :], T[:, 2:9:3, :])  # {3,6,9}
            chain(T[:, 10, :], V[:, 9, :], T[:, 9, :])
            chain(T[:, 11, :], V[:, 10, :], T[:, 8, :])
            chain(T[:, 14, :], V[:, 13, :], T[:, 8, :])
            chain(T[:, 12:16:3, :], V[:, 11:15:3, :], T[:, 11:15:3, :])  # {12,15}
            chain(T[:, 13:17:3, :], V[:, 12:16:3, :], T[:, 12:16:3, :])  # {13,16}

            # --- out = T + points (in place into the points tile), then store
            pf = Pt[:].rearrange("p a b -> p (a b)")
            nc.vector.tensor_add(pf, T[:].rearrange("p a b -> p (a b)"), pf)
            nc.gpsimd.dma_start(out.ap()[r0 : r0 + P], Pt[:])

        if rep_ctx is not None:
            rep_ctx.__exit__(None, None, None)

    nc.compile()
    return nc


def _get_nc():
    global _nc_cache
    if _nc_cache is None:
        _nc_cache = _build()
    return _nc_cache


def _make_in_maps(points, vectors, attntion_scors, Fa):
    points = np.ascontiguousarray(points, dtype=np.float32)
    vectors = np.ascontiguousarray(vectors, dtype=np.float32)
    attn = np.ascontiguousarray(attntion_scors, dtype=np.float32)
    fa = np.float32(np.asarray(Fa).reshape(-1)[0])
    fa_pos = np.full((P, 1), fa, dtype=np.float32)
    fa_neg = np.full((P, 1), -fa, dtype=np.float32)
    iota = np.tile(np.arange(NJ, dtype=np.float32), (P, 1))
    in_maps = []
    for c in range(N_CORES):
        s = slice(c * B, (c + 1) * B)
        in_maps.append(
            {
                "points": points[s],
                "vectors": vectors[s],
                "attn": attn[s],
                "fa_pos": fa_pos,
                "fa_neg": fa_neg,
                "iota": iota,
            }
        )
    return in_maps


def run(points, vectors, attntion_scors, Fa, trace=False, **spmd_kwargs):
    nc = _get_nc()
    in_maps = _make_in_maps(points, vectors, attntion_scors, Fa)
    res = run_bass_kernel_spmd(
        nc, in_maps, core_ids=list(range(N_CORES)), trace=trace, **spmd_kwargs
    )
    full = np.concatenate([res.results[c]["out"] for c in range(N_CORES)], axis=0)
    return full, res


def kernel(points, vectors, attntion_scors, Fa):
    full, _ = run(points, vectors, attntion_scors, Fa)
    return full
